# revision 37
# baseline (speedup 1.0000x reference)
"""Multi-head attention (QKV proj + per-head RMSNorm + softmax attention +
output proj) for Trainium2, distributed over 8 NeuronCores.

Sharding: batch (2) x head-groups (4 heads per core).  Per core, for its batch
element and 4 heads (2 pairs):

- All matmuls run in bf16 with fp32 PSUM accumulation (fp8 was measured to
  cost ~3.5% output rms: softmax output is itself a weighted mean, so
  per-element P/V/Qhat quantization error does NOT average down over keys).
- Projections go through [128,512] PSUM tiles, evicted to bf16 SBUF staging.
- Per-head RMSNorm: sumsq via a [128,2] ones-block matmul; rstd is computed
  on GPSIMD (quake rsqrt seed + 2 sign-cancelling Newton steps) on a
  DMA-packed [128,32] tile so the DVE queue never blocks on the DMA
  round-trip; the result is DMA partition-broadcast and folded into Q/K by a
  DVE scalar_tensor_tensor multiply (all-bf16, 4x DVE mode).
- Attention in S^T = [key, query] layout: scores contract over 128 rows
  (64 real + 64 zero-padded, free since PE time only depends on free size),
  exp runs on ACT (ScalarE) from PSUM f32 to bf16 P tiles - ACT does only
  exp + a few same-table copies, no activation-table thrashing.  O^T
  accumulates with [V|1] weights so softmax denominators fall out of the
  same matmul (no extra pass).
- Denominator reciprocals: DMA-pack the 4 per-head rows into [128,32], DVE
  reciprocal_approx_accurate, DMA-broadcast back; O is normalized by a
  4x-mode DVE multiply.  The output projection runs per token-half so the
  first half overlaps the second half's attention.
- Loop order is query-half outer, head inner; pair-1 projections are emitted
  as fine-grained chunks interleaved under heads 0-1 of the first query half
  so TensorE (the global bottleneck at ~170us busy) never idles.
"""

import os
import sys

for _p in ("/opt/trn_rl_repo",):
    if _p not in sys.path:
        sys.path.insert(0, _p)

import numpy as np

B = 2
T = 2048
D = 1024
H = 16
HD = 64
HPC = 4          # heads per core
NPAIR = 2
N_CORES = 8
EPS = 1e-5
TT = T // 128    # 16 key tiles
CT = D // 128    # 8 contraction tiles
QH = T // 1024   # 2 query halves

_COMPILED = None
LAST_EXEC_NS = None


def _install_ntff_shim():
    """antenv.axon_hooks is missing in this image; provide it so that
    BASS_TRACE=1 profiling works (mirrors trn_boot's ctypes hook)."""
    import contextlib
    import ctypes
    import types

    if "antenv.axon_hooks" in sys.modules:
        return
    so_path = "/opt/axon/libaxon_pjrt.so"
    if not os.path.exists(so_path):
        return
    lib = ctypes.CDLL(so_path)
    if not hasattr(lib, "axon_start_nrt_profile"):
        return
    lib.axon_start_nrt_profile.argtypes = [ctypes.POINTER(ctypes.c_int64), ctypes.c_size_t]
    lib.axon_start_nrt_profile.restype = ctypes.c_int64
    lib.axon_stop_nrt_profile.argtypes = [ctypes.c_char_p]
    lib.axon_stop_nrt_profile.restype = ctypes.c_int64

    @contextlib.contextmanager
    def _hook(output_dir, device_ids):
        import jax

        jax.devices()
        if device_ids:
            ids = (ctypes.c_int64 * len(device_ids))(*device_ids)
            rc = lib.axon_start_nrt_profile(ids, len(device_ids))
        else:
            rc = lib.axon_start_nrt_profile(None, 0)
        if rc != 0:
            raise RuntimeError(f"axon_start_nrt_profile rc={rc}")
        try:
            yield
        finally:
            n = lib.axon_stop_nrt_profile(str(output_dir).encode())
            print(f"profile: {n} file(s) written to {output_dir}", file=sys.stderr)

    mod = types.ModuleType("antenv.axon_hooks")
    mod._hook = _hook
    mod.get_axon_ntff_profile_hook = lambda: mod._hook
    mod.set_axon_ntff_profile_hook = lambda h: setattr(mod, "_hook", h)
    sys.modules["antenv.axon_hooks"] = mod
    try:
        import antenv

        antenv.axon_hooks = mod
    except ImportError:
        pass


def _build():
    import concourse.bass as bass
    import concourse.tile as tile
    from concourse import bacc, mybir

    F32 = mybir.dt.float32
    BF16 = mybir.dt.bfloat16
    U32 = mybir.dt.uint32
    Exp = mybir.ActivationFunctionType.Exp
    Ln = (mybir.ActivationFunctionType.Ln if hasattr(mybir.ActivationFunctionType, "Ln")
          else mybir.ActivationFunctionType.Log)
    mult = mybir.AluOpType.mult
    add = mybir.AluOpType.add
    sub = mybir.AluOpType.subtract
    bxor = mybir.AluOpType.bitwise_xor
    shr = mybir.AluOpType.logical_shift_right
    bypass = mybir.AluOpType.bypass

    nc = bacc.Bacc("TRN2", target_bir_lowering=False, debug=False, num_devices=N_CORES)

    xbT = nc.dram_tensor("xbT", (D, T), BF16, kind="ExternalInput").ap()
    wq_s = nc.dram_tensor("wq_s", (D, HPC * HD), BF16, kind="ExternalInput").ap()
    wk_s = nc.dram_tensor("wk_s", (D, HPC * HD), BF16, kind="ExternalInput").ap()
    wv_s = nc.dram_tensor("wv_s", (D, HPC * HD), BF16, kind="ExternalInput").ap()
    wo_s = nc.dram_tensor("wo_s", (HPC * HD, D), BF16, kind="ExternalInput").ap()
    ident_d = nc.dram_tensor("ident", (128, 128), BF16, kind="ExternalInput").ap()
    bd2_d = nc.dram_tensor("bd2", (128, 2), BF16, kind="ExternalInput").ap()
    wqc_d = nc.dram_tensor("wqc", (128, 1), F32, kind="ExternalInput").ap()
    wkc_d = nc.dram_tensor("wkc", (128, 1), F32, kind="ExternalInput").ap()
    sel_d = [nc.dram_tensor(f"sel{p}", (128, 128), BF16, kind="ExternalInput").ap()
             for p in range(NPAIR)]
    outT = nc.dram_tensor("outT", (D, T), BF16, kind="ExternalOutput").ap()
    outT2 = nc.dram_tensor("outT2", (D, 1024), BF16, kind="ExternalOutput").ap()

    def dram_view(tl, shape):
        """raw row-major AP view over a DRAM tile's buffer"""
        ap = tl[:]
        strides = []
        s = 1
        for n in reversed(shape):
            strides.append([s, n])
            s *= n
        return bass.AP(tensor=ap.tensor, offset=ap.offset, ap=list(reversed(strides)))

    def bcast_row(row_ap, n=64):
        """partition-broadcast a [1, cols] AP to n partitions"""
        return bass.AP(tensor=row_ap.tensor, offset=row_ap.offset,
                       ap=[[0, n]] + list(row_ap.ap[1:]))

    with tile.TileContext(nc) as tc:
        from contextlib import ExitStack

        with ExitStack() as top:
            consts = top.enter_context(tc.tile_pool(name="consts", bufs=1))
            xtp = top.enter_context(tc.tile_pool(name="xT", bufs=1))
            wpool = top.enter_context(tc.tile_pool(name="w", bufs=1))
            qtp = top.enter_context(tc.tile_pool(name="qt", bufs=1))
            q2p = top.enter_context(tc.tile_pool(name="q2", bufs=2))
            msp = top.enter_context(tc.tile_pool(name="ms", bufs=1))
            hatp = top.enter_context(tc.tile_pool(name="hat", bufs=1))
            vpp = top.enter_context(tc.tile_pool(name="vp", bufs=1))
            pbp = top.enter_context(tc.tile_pool(name="pb", bufs=5))
            dntp = top.enter_context(tc.tile_pool(name="dnt", bufs=2))
            ohpp = top.enter_context(tc.tile_pool(name="ohp", bufs=1))
            rwp = top.enter_context(tc.tile_pool(name="rw", bufs=2))
            packp = top.enter_context(tc.tile_pool(name="pack", bufs=1))
            osbp = top.enter_context(tc.tile_pool(name="osb", bufs=3))
            psS = top.enter_context(tc.tile_pool(name="psS", bufs=2, space="PSUM"))
            psO = top.enter_context(tc.tile_pool(name="psO", bufs=1, space="PSUM"))
            psX = top.enter_context(tc.tile_pool(name="psX", bufs=2, space="PSUM"))
            drp = top.enter_context(tc.tile_pool(name="dr", bufs=1, space="DRAM"))

            # qhat/khat[h]: [128, T] bf16; head h=2p+i data in rows 64i:64i+64,
            # complement rows must be zero (contraction zero-padding)
            qhat = [hatp.tile([128, T], BF16, tag=f"qh{h}", name=f"qhat{h}")
                    for h in range(HPC)]
            khat = [hatp.tile([128, T], BF16, tag=f"kh{h}", name=f"khat{h}")
                    for h in range(HPC)]
            for h in range(HPC):
                i = h % 2
                rows = slice(64 * (1 - i), 64 * (1 - i) + 64)
                nc.gpsimd.memset(qhat[h][rows, :], 0.0)
                nc.gpsimd.memset(khat[h][rows, :], 0.0)
            # vp[pair]: [128 keys, kt, head-in-pair, 65] bf16; col 64 = 1.0
            vp = [vpp.tile([128, TT, 2, 65], BF16, tag=f"v{p}", name=f"vp{p}")
                  for p in range(NPAIR)]
            for p in range(NPAIR):
                nc.vector.memset(vp[p][:, :, :, 64:65], 1.0)

            # ---------------- input DMA (consumption order) -------------------
            wk_sb = [wpool.tile([128, 256], BF16, tag=f"wk{c}", name=f"wk{c}")
                     for c in range(CT)]
            wq_sb = [wpool.tile([128, 256], BF16, tag=f"wq{c}", name=f"wq{c}")
                     for c in range(CT)]
            wv_sb = [wpool.tile([128, 256], BF16, tag=f"wv{c}", name=f"wv{c}")
                     for c in range(CT)]
            xT = [xtp.tile([128, T], BF16, tag=f"xT{c}", name=f"xT{c}")
                  for c in range(CT)]
            for c in range(CT):
                nc.sync.dma_start(out=wk_sb[c][:], in_=wk_s[c * 128:(c + 1) * 128, :])
                nc.sync.dma_start(out=wq_sb[c][:], in_=wq_s[c * 128:(c + 1) * 128, :])
                nc.sync.dma_start(out=xT[c][:, 0:1024],
                                  in_=xbT[c * 128:(c + 1) * 128, 0:1024])
            for c in range(CT):
                nc.sync.dma_start(out=xT[c][:, 1024:2048],
                                  in_=xbT[c * 128:(c + 1) * 128, 1024:2048])
            for c in range(CT):
                nc.sync.dma_start(out=wv_sb[c][:], in_=wv_s[c * 128:(c + 1) * 128, :])
            wo_sb = []
            for cp in range(NPAIR):
                wt = wpool.tile([128, D], BF16, tag=f"wo{cp}", name=f"wo{cp}")
                nc.gpsimd.dma_start(out=wt[:], in_=wo_s[cp * 128:(cp + 1) * 128, :])
                wo_sb.append(wt)
            ident = consts.tile([128, 128], BF16, tag="ident")
            nc.gpsimd.dma_start(out=ident[:], in_=ident_d)
            bd2 = consts.tile([128, 2], BF16, tag="bd2")
            nc.gpsimd.dma_start(out=bd2[:], in_=bd2_d)
            wqc = consts.tile([128, 1], F32, tag="wqc")
            nc.gpsimd.dma_start(out=wqc[:], in_=wqc_d)
            wkc = consts.tile([128, 1], F32, tag="wkc")
            nc.gpsimd.dma_start(out=wkc[:], in_=wkc_d)
            epsc = consts.tile([128, 1], F32, tag="epsc")
            nc.vector.memset(epsc[:], EPS)
            sel_sb = []
            for p in range(NPAIR):
                st = consts.tile([128, 128], BF16, tag=f"sel{p}", name=f"sel{p}")
                nc.gpsimd.dma_start(out=st[:], in_=sel_d[p])
                sel_sb.append(st)

            qt = {}      # (proj, pair) -> [128, T] bf16 staging
            ms = {}      # (proj, pair) -> [2, T] bf16 sumsq rows
            vt_sb = {}   # pair -> [128, T] bf16 V^T staging
            rstdD = {}   # (proj, pair) -> DRAM [2, T] bf16
            msD = {}
            for proj in ("k", "q"):
                for p in range(NPAIR):
                    qt[(proj, p)] = qtp.tile([128, T], BF16, tag=f"qt{proj}{p}",
                                             name=f"qt{proj}{p}")
                    ms[(proj, p)] = msp.tile([2, T], BF16, tag=f"ms{proj}{p}",
                                             name=f"ms{proj}{p}")
                    for qh in range(QH):
                        msD[(proj, p, qh)] = drp.tile(
                            [2, 1024], BF16, tag=f"msD{proj}{p}{qh}",
                            name=f"msD{proj}{p}{qh}")
                        rstdD[(proj, p, qh)] = drp.tile(
                            [2, 1024], BF16, tag=f"rsD{proj}{p}{qh}",
                            name=f"rsD{proj}{p}{qh}")
            for p in range(NPAIR):
                vt_sb[p] = qtp.tile([128, T], BF16, tag=f"vt{p}", name=f"vt{p}")

            ohp = [ohpp.tile([128, T], BF16, tag=f"ohp{p}", name=f"ohp{p}")
                   for p in range(NPAIR)]
            ohr = [ohpp.tile([128, T], BF16, tag=f"ohr{p}", name=f"ohr{p}")
                   for p in range(NPAIR)]


            W_SB = {"k": wk_sb, "q": wq_sb, "v": wv_sb}

            # ---------------- building blocks ---------------------------------
            def proj_chunk(proj, pair, qh, qq):
                dest = vt_sb[pair] if proj == "v" else qt[(proj, pair)]
                pj = psX.tile([128, 512], F32, tag="x")
                off = qh * 1024 + qq * 512
                for ct in range(CT):
                    nc.tensor.matmul(
                        pj[:], W_SB[proj][ct][:, pair * 128:(pair + 1) * 128],
                        xT[ct][:, off:off + 512],
                        start=(ct == 0), stop=(ct == CT - 1))
                with nc.allow_low_precision(reason="bf16 staging"):
                    nc.vector.tensor_copy(out=dest[:, off:off + 512], in_=pj[:])

            def sumsq_chunk(proj, pair, qh):
                q2 = q2p.tile([128, 1024], BF16, tag="q2")
                sl = slice(qh * 1024, (qh + 1) * 1024)
                with nc.allow_low_precision(reason="bf16 sumsq"):
                    nc.vector.tensor_tensor(out=q2[:], in0=qt[(proj, pair)][:, sl],
                                            in1=qt[(proj, pair)][:, sl], op=mult)
                for qq in range(2):
                    ss = psX.tile([128, 512], F32, tag="x")
                    nc.tensor.matmul(ss[0:2, :], bd2[:], q2[:, qq * 512:(qq + 1) * 512],
                                     start=True, stop=True)
                    # ACT is idle pre-attention; Copy lives in every act table
                    with nc.allow_low_precision(reason="bf16 ms"):
                        nc.scalar.copy(
                            ms[(proj, pair)][:, qh * 1024 + qq * 512:
                                             qh * 1024 + (qq + 1) * 512],
                            ss[0:2, :])

            def rstd_unit(proj, pair, qh):
                """rstdD = (ms/64+eps)^-1/2 for one query half, DMA-packed
                into [64,32] so the DVE chain is cheap"""
                sl = slice(qh * 1024, (qh + 1) * 1024)
                nm = f"{proj}{pair}{qh}"
                nc.sync.dma_start(out=msD[(proj, pair, qh)][:],
                                  in_=ms[(proj, pair)][:, sl])
                mp = packp.tile([64, 32], BF16, tag="pk16", name=f"mp{nm}")
                nc.sync.dma_start(out=mp[:], in_=dram_view(msD[(proj, pair, qh)], [64, 32]))
                m = packp.tile([64, 32], F32, tag="pk32a", name=f"m{nm}")
                mh = packp.tile([64, 32], F32, tag="pk32b", name=f"mh{nm}")
                y = packp.tile([64, 32], F32, tag="pk32c", name=f"y{nm}")
                t1 = packp.tile([64, 32], F32, tag="pk32d", name=f"t1{nm}")
                t2 = packp.tile([64, 32], F32, tag="pk32e", name=f"t2{nm}")
                rs = packp.tile([64, 32], BF16, tag="pk16b", name=f"rs{nm}")
                ts = nc.vector.tensor_scalar
                tt = nc.vector.tensor_tensor
                with nc.allow_low_precision(reason="rstd chain"):
                    ts(out=m[:], in0=mp[:], scalar1=1.0 / HD, scalar2=EPS,
                       op0=mult, op1=add)
                    ts(out=mh[:], in0=m[:], scalar1=0.5, scalar2=0.0,
                       op0=mult, op1=bypass)
                    # y0 = bitcast(((~bits) >> 1) - 0x20A8C620)
                    ts(out=y[:].bitcast(U32), in0=m[:].bitcast(U32),
                       scalar1=0xFFFFFFFF, scalar2=1, op0=bxor, op1=shr)
                    ts(out=y[:].bitcast(U32), in0=y[:].bitcast(U32),
                       scalar1=0x20A8C620, scalar2=0, op0=sub, op1=bypass)
                    for _ in range(2):  # two sign-cancelling Newton steps
                        tt(out=t1[:], in0=mh[:], in1=y[:], op=mult)
                        tt(out=t2[:], in0=t1[:], in1=y[:], op=mult)
                        ts(out=t2[:], in0=t2[:], scalar1=1.5, scalar2=0.0,
                           op0=sub, op1=bypass)
                        tt(out=y[:], in0=t2[:], in1=y[:], op=mult)
                    nc.vector.tensor_copy(out=rs[:], in_=y[:])
                nc.sync.dma_start(out=dram_view(rstdD[(proj, pair, qh)], [64, 32]),
                                  in_=rs[:])

            def rstd_unit_act(proj, pair, qh):
                """pair-0 warmup variant: rstd on the idle ACT engine via
                Ln/Exp in [2,1024] layout - skips the DMA pack round-trip.
                Must be emitted before the first attention exp (table order)."""
                sl = slice(qh * 1024, (qh + 1) * 1024)
                nm = f"{proj}{pair}{qh}"
                lnt = packp.tile([2, 1024], F32, tag="lnt", name=f"ln{nm}")
                rsa = packp.tile([2, 1024], BF16, tag="rsa", name=f"rsa{nm}")
                nc.scalar.activation(lnt[:], ms[(proj, pair)][:, sl], Ln,
                                     scale=1.0 / HD, bias=epsc[0:2, :])
                with nc.allow_low_precision(reason="bf16 rstd"):
                    nc.scalar.activation(rsa[:], lnt[:], Exp, scale=-0.5)
                nc.sync.dma_start(out=rstdD[(proj, pair, qh)][:], in_=rsa[:])

            def norm_chunk(proj, pair, qh):
                """qhat/khat[2p+i][64i:64i+64, :] = qt * wcol * rstd (bf16)"""
                dest = qhat if proj == "q" else khat
                wcol = wqc if proj == "q" else wkc
                rw = rwp.tile([128, 1024], BF16, tag="rwn")
                sl = slice(qh * 1024, (qh + 1) * 1024)
                for i in range(2):
                    nc.sync.dma_start(
                        out=rw[64 * i:64 * i + 64, :],
                        in_=bcast_row(rstdD[(proj, pair, qh)][i:i + 1, :]))
                with nc.allow_low_precision(reason="bf16 qkhat"):
                    for i in range(2):
                        rows = slice(64 * i, 64 * i + 64)
                        nc.vector.scalar_tensor_tensor(
                            out=dest[pair * 2 + i][rows, sl],
                            in0=qt[(proj, pair)][rows, sl],
                            scalar=wcol[rows, :],
                            in1=rw[rows, :],
                            op0=mult, op1=mult)

            def vtrans_chunk(pair, tq):
                """vt_sb[pair] kt 4tq..4tq+3 -> vp[pair] bf16 [V|1] weights"""
                vb = psX.tile([128, 512], F32, tag="x")
                vbb = vb[:].bitcast(BF16)  # [128, 1024] bf16 view
                for b_ in range(4):
                    kt = tq * 4 + b_
                    nc.tensor.transpose(vbb[:, b_ * 128:(b_ + 1) * 128],
                                        vt_sb[pair][:, kt * 128:(kt + 1) * 128],
                                        ident[:])
                for b_ in range(4):
                    kt = tq * 4 + b_
                    with nc.allow_low_precision(reason="bf16 v"):
                        nc.vector.tensor_copy(
                            out=vp[pair][:, kt, :, 0:64],
                            in_=vbb[:, b_ * 128:(b_ + 1) * 128].rearrange(
                                "p (h d) -> p h d", h=2))

            # ---------------- pre-attention: pair-0 K/Q pipeline --------------
            # per query-half so the first scores tile is gated by only half of
            # the projection work; V and its transposes hide under attention
            for qh in range(QH):
                for qq in range(2):
                    proj_chunk("k", 0, qh, qq)
                sumsq_chunk("k", 0, qh)
                rstd_unit_act("k", 0, qh)
                for qq in range(2):
                    proj_chunk("q", 0, qh, qq)
                sumsq_chunk("q", 0, qh)
                rstd_unit_act("q", 0, qh)
                norm_chunk("k", 0, qh)
                norm_chunk("q", 0, qh)
            for qh in range(QH):
                for qq in range(2):
                    proj_chunk("v", 0, qh, qq)

            # pair-1 work, finely chunked, interleaved under heads 0-1 of the
            # first query half; fully drained before head 2 is emitted
            bg = []
            for proj in ("k", "q"):
                for qh in range(QH):
                    for qq in range(2):
                        bg.append(lambda proj=proj, qh=qh, qq=qq:
                                  proj_chunk(proj, 1, qh, qq))
                    bg.append(lambda proj=proj, qh=qh: sumsq_chunk(proj, 1, qh))
                    bg.append(lambda proj=proj, qh=qh: rstd_unit(proj, 1, qh))
            for proj in ("k", "q"):
                for qh in range(QH):
                    bg.append(lambda proj=proj, qh=qh: norm_chunk(proj, 1, qh))
            for qh in range(QH):
                for qq in range(2):
                    bg.append(lambda qh=qh, qq=qq: proj_chunk("v", 1, qh, qq))
            for tq in range(4):
                bg.append(lambda tq=tq: vtrans_chunk(1, tq))

            def pop_bg(n=1):
                for _ in range(n):
                    if bg:
                        bg.pop(0)()

            # V-of-pair-0 transposes hide under head 0's first score tiles;
            # its O matmuls lag 4 tiles behind the exps
            bg0 = []
            for tq in range(4):
                bg0.append(lambda tq=tq: vtrans_chunk(0, tq))

            # ---------------- attention + output projection -------------------
            # pair-outer head order: pair-1's projection chain hides under all
            # four pair-0 (head, half) tiles instead of just two; the qh0
            # finalization (recip + output projection) pops inside the last
            # two heads' loops so it never stalls the PE queue
            dns = {(qh, p): packp.tile([2, 1024], BF16, tag=f"dns{qh}{p}",
                                       name=f"dns{qh}{p}")
                   for qh in range(QH) for p in range(NPAIR)}

            def recip_ohr(qh, pair):
                nm = f"{qh}{pair}"
                dnf = packp.tile([2, 1024], F32, tag="dnf", bufs=2, name=f"dnf{nm}")
                rcf = packp.tile([2, 1024], F32, tag="rcf", bufs=2, name=f"rcf{nm}")
                rc16 = packp.tile([128, 1024], BF16, tag="rc16", bufs=2,
                                  name=f"rc16{nm}")
                nc.gpsimd.memset(rc16[:], 0.0)
                with nc.allow_low_precision(reason="recip"):
                    nc.vector.tensor_copy(out=dnf[:], in_=dns[(qh, pair)][:])
                    nc.vector.reciprocal_approx_fast(out=rcf[:], in_=dnf[:])
                    nc.vector.tensor_copy(out=rc16[0:2, :], in_=rcf[:])
                for qq in range(2):
                    rb = psX.tile([128, 512], F32, tag="x")
                    # sel_sb[0] maps rows 0/1 to partition blocks 0:64/64:128
                    nc.tensor.matmul(rb[:], sel_sb[0][:],
                                     rc16[:, qq * 512:(qq + 1) * 512],
                                     start=True, stop=True)
                    csl = slice(qh * 1024 + qq * 512, qh * 1024 + (qq + 1) * 512)
                    with nc.allow_low_precision(reason="bf16 ohat"):
                        nc.vector.tensor_tensor(out=ohr[pair][:, csl],
                                                in0=ohp[pair][:, csl],
                                                in1=rb[:], op=mult)

            def outproj_cp(cp, qh, et, dest, dcol):
                """single-pair output-projection partial for one et block"""
                for qq in range(2):
                    ops = psX.tile([128, 512], F32, tag="x")
                    nc.tensor.matmul(
                        ops[:], wo_sb[cp][:, et * 128:(et + 1) * 128],
                        ohr[cp][:, qh * 1024 + qq * 512:qh * 1024 + (qq + 1) * 512],
                        start=True, stop=True)
                    osb = osbp.tile([128, 512], BF16, tag="osb")
                    with nc.allow_low_precision(reason="bf16 out"):
                        if qq == 0:
                            nc.vector.tensor_copy(out=osb[:], in_=ops[:])
                        else:
                            nc.scalar.copy(osb[:], ops[:])
                    eng = (nc.gpsimd, nc.sync)[(et + qq) % 2]
                    eng.dma_start(
                        out=dest[et * 128:(et + 1) * 128,
                                 dcol + qq * 512:dcol + (qq + 1) * 512],
                        in_=osb[:])

            def outproj_psx(qh, et):
                for qq in range(2):
                    ops = psX.tile([128, 512], F32, tag="x")
                    for cp in range(NPAIR):
                        nc.tensor.matmul(
                            ops[:], wo_sb[cp][:, et * 128:(et + 1) * 128],
                            ohr[cp][:, qh * 1024 + qq * 512:
                                    qh * 1024 + (qq + 1) * 512],
                            start=(cp == 0), stop=(cp == NPAIR - 1))
                    osb = osbp.tile([128, 512], BF16, tag="osb")
                    with nc.allow_low_precision(reason="bf16 out"):
                        if qq == 0:
                            nc.vector.tensor_copy(out=osb[:], in_=ops[:])
                        else:
                            nc.scalar.copy(osb[:], ops[:])
                    eng = (nc.gpsimd, nc.sync)[(et + qq) % 2]
                    eng.dma_start(
                        out=outT[et * 128:(et + 1) * 128,
                                 qh * 1024 + qq * 512:qh * 1024 + (qq + 1) * 512],
                        in_=osb[:])

            bg2 = []
            bg3 = []
            seq = [(0, 0), (1, 0), (0, 1), (1, 1), (2, 0), (3, 0), (2, 1), (3, 1)]
            for idx, (h, qh) in enumerate(seq):
                pair, i = h // 2, h % 2
                lag = 4 if idx == 0 else 1
                o_ps = psO.tile([65, 1024], F32, tag="o")
                pbs = {}

                def emit_o(kt, o_ps=o_ps, pbs=pbs, pair=pair, i=i):
                    for qq in range(2):
                        nc.tensor.matmul(
                            o_ps[:, qq * 512:(qq + 1) * 512],
                            vp[pair][:, kt, i, :],
                            pbs[kt][:, qq * 512:(qq + 1) * 512],
                            start=(kt == 0), stop=(kt == TT - 1))

                for kt in range(TT):
                    s_ps = psS.tile([128, 1024], F32, tag="s")
                    for qq in range(2):
                        nc.tensor.matmul(
                            s_ps[:, qq * 512:(qq + 1) * 512],
                            khat[h][:, kt * 128:(kt + 1) * 128],
                            qhat[h][:, qh * 1024 + qq * 512:
                                    qh * 1024 + (qq + 1) * 512],
                            start=True, stop=True)
                    pb = pbp.tile([128, 1024], BF16, tag="pb")
                    pbs[kt] = pb
                    nc.scalar.activation(pb[:], s_ps[:], Exp, scale=0.125)
                    if idx == 0 and bg0:
                        bg0.pop(0)()
                    if kt >= lag:
                        emit_o(kt - lag)
                    if pair == 0 and not bg0 and kt % 2 == 0:
                        pop_bg(1)
                    if idx in (4, 5) and bg3:
                        bg3.pop(0)()
                    if idx >= 6 and bg2:
                        bg2.pop(0)()
                for kt in range(TT - lag, TT):
                    emit_o(kt)
                # evict unnormalized O + denominator row
                sl = slice(qh * 1024, (qh + 1) * 1024)
                dnt = dntp.tile([65, 1024], BF16, tag="dnt")
                with nc.allow_low_precision(reason="bf16 o"):
                    nc.vector.tensor_copy(out=dnt[:], in_=o_ps[:])
                nc.gpsimd.dma_start(out=dns[(qh, pair)][i:i + 1, :],
                                    in_=dnt[64:65, :])
                nc.gpsimd.dma_start(out=ohp[pair][64 * i:64 * i + 64, sl],
                                    in_=dnt[0:64, :])
                if idx == 3:
                    pop_bg(len(bg))  # pair-1 data must exist before head 2
                    bg3.append(lambda: recip_ohr(1, 0))
                    for et in range(D // 128):
                        bg3.append(lambda et=et: outproj_cp(0, 1, et, outT, 1024))
                if idx == 5:
                    bg2.append(lambda: recip_ohr(0, 0))
                    bg2.append(lambda: recip_ohr(0, 1))
                    for et in range(D // 128):
                        bg2.append(lambda et=et: outproj_psx(0, et))
            while bg3:
                bg3.pop(0)()
            while bg2:
                bg2.pop(0)()
            # tail: only pair-1's second-half partial remains (summed on host)
            recip_ohr(1, 1)
            for et in range(D // 128):
                outproj_cp(1, 1, et, outT2, 0)

    nc.compile()
    return nc


def _get_compiled():
    global _COMPILED
    if _COMPILED is None:
        _COMPILED = _build()
    return _COMPILED


def kernel(x, wq, wk, wv, wo, q_norm_w, k_norm_w):
    import ml_dtypes
    from concourse.bass_utils import run_bass_kernel_spmd

    global LAST_EXEC_NS
    if os.environ.get("BASS_TRACE"):
        _install_ntff_shim()

    x = np.asarray(x, dtype=np.float32)
    wq = np.asarray(wq, dtype=np.float32)
    wk = np.asarray(wk, dtype=np.float32)
    wv = np.asarray(wv, dtype=np.float32)
    wo = np.asarray(wo, dtype=np.float32)
    q_norm_w = np.asarray(q_norm_w, dtype=np.float32)
    k_norm_w = np.asarray(k_norm_w, dtype=np.float32)

    nc = _get_compiled()
    bf = lambda a: a.astype(ml_dtypes.bfloat16)

    ident = np.eye(128, dtype=np.float32)
    bd2 = np.zeros((128, 2), np.float32)
    bd2[0:64, 0] = 1.0
    bd2[64:128, 1] = 1.0
    wqc = np.concatenate([q_norm_w, q_norm_w]).reshape(128, 1).astype(np.float32)
    wkc = np.concatenate([k_norm_w, k_norm_w]).reshape(128, 1).astype(np.float32)
    sels = []
    for p in range(NPAIR):
        sm = np.zeros((128, 128), np.float32)
        sm[2 * p, 0:64] = 1.0
        sm[2 * p + 1, 64:128] = 1.0
        sels.append(sm)

    in_maps = []
    for c in range(N_CORES):
        b = c // 4
        hs = HPC * (c % 4)
        # head split in reference is strided: head h uses columns d*H + h
        perm = ((hs + np.arange(HPC))[:, None] + H * np.arange(HD)[None, :]).reshape(-1)
        in_maps.append({
            "xbT": bf(np.ascontiguousarray(x[b].T)),
            "wq_s": bf(np.ascontiguousarray(wq[:, perm])),
            "wk_s": bf(np.ascontiguousarray(wk[:, perm])),
            "wv_s": bf(np.ascontiguousarray(wv[:, perm])),
            "wo_s": bf(np.ascontiguousarray(wo[hs * HD:(hs + HPC) * HD, :])),
            "ident": bf(ident), "bd2": bf(bd2),
            "wqc": wqc, "wkc": wkc,
            "sel0": bf(sels[0]), "sel1": bf(sels[1]),
        })

    res = run_bass_kernel_spmd(nc, in_maps, core_ids=list(range(N_CORES)),
                               trace=bool(os.environ.get("BASS_TRACE")),
                               tmpdir=os.environ.get("BASS_TRACE_DIR"))
    LAST_EXEC_NS = res.exec_time_ns

    out = np.empty((B, T, D), dtype=np.float32)
    for b in range(B):
        acc = res.results[4 * b]["outT"].astype(np.float32)
        acc[:, 1024:] += res.results[4 * b]["outT2"].astype(np.float32)
        for c in range(4 * b + 1, 4 * b + 4):
            acc = acc + res.results[c]["outT"].astype(np.float32)
            acc[:, 1024:] += res.results[c]["outT2"].astype(np.float32)
        out[b] = acc.T
    return out


# revision 38
# speedup vs baseline: 1.0042x; 1.0042x over previous
"""Multi-head attention (QKV proj + per-head RMSNorm + softmax attention +
output proj) for Trainium2, distributed over 8 NeuronCores.

Sharding: batch (2) x head-groups (4 heads per core).  Per core, for its batch
element and 4 heads (2 pairs):

- All matmuls run in bf16 with fp32 PSUM accumulation (fp8 was measured to
  cost ~3.5% output rms: softmax output is itself a weighted mean, so
  per-element P/V/Qhat quantization error does NOT average down over keys).
- Projections go through [128,512] PSUM tiles, evicted to bf16 SBUF staging.
- Per-head RMSNorm: sumsq via a [128,2] ones-block matmul; rstd is computed
  on GPSIMD (quake rsqrt seed + 2 sign-cancelling Newton steps) on a
  DMA-packed [128,32] tile so the DVE queue never blocks on the DMA
  round-trip; the result is DMA partition-broadcast and folded into Q/K by a
  DVE scalar_tensor_tensor multiply (all-bf16, 4x DVE mode).
- Attention in S^T = [key, query] layout: scores contract over 128 rows
  (64 real + 64 zero-padded, free since PE time only depends on free size),
  exp runs on ACT (ScalarE) from PSUM f32 to bf16 P tiles - ACT does only
  exp + a few same-table copies, no activation-table thrashing.  O^T
  accumulates with [V|1] weights so softmax denominators fall out of the
  same matmul (no extra pass).
- Denominator reciprocals: DMA-pack the 4 per-head rows into [128,32], DVE
  reciprocal_approx_accurate, DMA-broadcast back; O is normalized by a
  4x-mode DVE multiply.  The output projection runs per token-half so the
  first half overlaps the second half's attention.
- Loop order is query-half outer, head inner; pair-1 projections are emitted
  as fine-grained chunks interleaved under heads 0-1 of the first query half
  so TensorE (the global bottleneck at ~170us busy) never idles.
"""

import os
import sys

for _p in ("/opt/trn_rl_repo",):
    if _p not in sys.path:
        sys.path.insert(0, _p)

import numpy as np

B = 2
T = 2048
D = 1024
H = 16
HD = 64
HPC = 4          # heads per core
NPAIR = 2
N_CORES = 8
EPS = 1e-5
TT = T // 128    # 16 key tiles
CT = D // 128    # 8 contraction tiles
QH = T // 1024   # 2 query halves

_COMPILED = None
LAST_EXEC_NS = None


def _install_ntff_shim():
    """antenv.axon_hooks is missing in this image; provide it so that
    BASS_TRACE=1 profiling works (mirrors trn_boot's ctypes hook)."""
    import contextlib
    import ctypes
    import types

    if "antenv.axon_hooks" in sys.modules:
        return
    so_path = "/opt/axon/libaxon_pjrt.so"
    if not os.path.exists(so_path):
        return
    lib = ctypes.CDLL(so_path)
    if not hasattr(lib, "axon_start_nrt_profile"):
        return
    lib.axon_start_nrt_profile.argtypes = [ctypes.POINTER(ctypes.c_int64), ctypes.c_size_t]
    lib.axon_start_nrt_profile.restype = ctypes.c_int64
    lib.axon_stop_nrt_profile.argtypes = [ctypes.c_char_p]
    lib.axon_stop_nrt_profile.restype = ctypes.c_int64

    @contextlib.contextmanager
    def _hook(output_dir, device_ids):
        import jax

        jax.devices()
        if device_ids:
            ids = (ctypes.c_int64 * len(device_ids))(*device_ids)
            rc = lib.axon_start_nrt_profile(ids, len(device_ids))
        else:
            rc = lib.axon_start_nrt_profile(None, 0)
        if rc != 0:
            raise RuntimeError(f"axon_start_nrt_profile rc={rc}")
        try:
            yield
        finally:
            n = lib.axon_stop_nrt_profile(str(output_dir).encode())
            print(f"profile: {n} file(s) written to {output_dir}", file=sys.stderr)

    mod = types.ModuleType("antenv.axon_hooks")
    mod._hook = _hook
    mod.get_axon_ntff_profile_hook = lambda: mod._hook
    mod.set_axon_ntff_profile_hook = lambda h: setattr(mod, "_hook", h)
    sys.modules["antenv.axon_hooks"] = mod
    try:
        import antenv

        antenv.axon_hooks = mod
    except ImportError:
        pass


def _build():
    import concourse.bass as bass
    import concourse.tile as tile
    from concourse import bacc, mybir

    F32 = mybir.dt.float32
    BF16 = mybir.dt.bfloat16
    U32 = mybir.dt.uint32
    Exp = mybir.ActivationFunctionType.Exp
    Ln = (mybir.ActivationFunctionType.Ln if hasattr(mybir.ActivationFunctionType, "Ln")
          else mybir.ActivationFunctionType.Log)
    mult = mybir.AluOpType.mult
    add = mybir.AluOpType.add
    sub = mybir.AluOpType.subtract
    bxor = mybir.AluOpType.bitwise_xor
    shr = mybir.AluOpType.logical_shift_right
    bypass = mybir.AluOpType.bypass

    nc = bacc.Bacc("TRN2", target_bir_lowering=False, debug=False, num_devices=N_CORES)

    xbT = nc.dram_tensor("xbT", (D, T), BF16, kind="ExternalInput").ap()
    wq_s = nc.dram_tensor("wq_s", (D, HPC * HD), BF16, kind="ExternalInput").ap()
    wk_s = nc.dram_tensor("wk_s", (D, HPC * HD), BF16, kind="ExternalInput").ap()
    wv_s = nc.dram_tensor("wv_s", (D, HPC * HD), BF16, kind="ExternalInput").ap()
    wo_s = nc.dram_tensor("wo_s", (HPC * HD, D), BF16, kind="ExternalInput").ap()
    ident_d = nc.dram_tensor("ident", (128, 128), BF16, kind="ExternalInput").ap()
    bd2_d = nc.dram_tensor("bd2", (128, 2), BF16, kind="ExternalInput").ap()
    wqc_d = nc.dram_tensor("wqc", (128, 1), F32, kind="ExternalInput").ap()
    wkc_d = nc.dram_tensor("wkc", (128, 1), F32, kind="ExternalInput").ap()
    sel_d = [nc.dram_tensor(f"sel{p}", (128, 128), BF16, kind="ExternalInput").ap()
             for p in range(NPAIR)]
    outT = nc.dram_tensor("outT", (D, T), BF16, kind="ExternalOutput").ap()

    def dram_view(tl, shape):
        """raw row-major AP view over a DRAM tile's buffer"""
        ap = tl[:]
        strides = []
        s = 1
        for n in reversed(shape):
            strides.append([s, n])
            s *= n
        return bass.AP(tensor=ap.tensor, offset=ap.offset, ap=list(reversed(strides)))

    def bcast_row(row_ap, n=64):
        """partition-broadcast a [1, cols] AP to n partitions"""
        return bass.AP(tensor=row_ap.tensor, offset=row_ap.offset,
                       ap=[[0, n]] + list(row_ap.ap[1:]))

    with tile.TileContext(nc) as tc:
        from contextlib import ExitStack

        with ExitStack() as top:
            consts = top.enter_context(tc.tile_pool(name="consts", bufs=1))
            xtp = top.enter_context(tc.tile_pool(name="xT", bufs=1))
            wpool = top.enter_context(tc.tile_pool(name="w", bufs=1))
            qtp = top.enter_context(tc.tile_pool(name="qt", bufs=1))
            q2p = top.enter_context(tc.tile_pool(name="q2", bufs=2))
            msp = top.enter_context(tc.tile_pool(name="ms", bufs=1))
            hatp = top.enter_context(tc.tile_pool(name="hat", bufs=1))
            vpp = top.enter_context(tc.tile_pool(name="vp", bufs=1))
            pbp = top.enter_context(tc.tile_pool(name="pb", bufs=5))
            dntp = top.enter_context(tc.tile_pool(name="dnt", bufs=2))
            ohpp = top.enter_context(tc.tile_pool(name="ohp", bufs=1))
            rwp = top.enter_context(tc.tile_pool(name="rw", bufs=2))
            packp = top.enter_context(tc.tile_pool(name="pack", bufs=1))
            osbp = top.enter_context(tc.tile_pool(name="osb", bufs=3))
            psS = top.enter_context(tc.tile_pool(name="psS", bufs=2, space="PSUM"))
            psO = top.enter_context(tc.tile_pool(name="psO", bufs=1, space="PSUM"))
            psX = top.enter_context(tc.tile_pool(name="psX", bufs=2, space="PSUM"))
            drp = top.enter_context(tc.tile_pool(name="dr", bufs=1, space="DRAM"))

            # qhat/khat[h]: [128, T] bf16; head h=2p+i data in rows 64i:64i+64,
            # complement rows must be zero (contraction zero-padding)
            qhat = [hatp.tile([128, T], BF16, tag=f"qh{h}", name=f"qhat{h}")
                    for h in range(HPC)]
            khat = [hatp.tile([128, T], BF16, tag=f"kh{h}", name=f"khat{h}")
                    for h in range(HPC)]
            for h in range(HPC):
                i = h % 2
                rows = slice(64 * (1 - i), 64 * (1 - i) + 64)
                nc.gpsimd.memset(qhat[h][rows, :], 0.0)
                nc.gpsimd.memset(khat[h][rows, :], 0.0)
            # vp[pair]: [128 keys, kt, head-in-pair, 65] bf16; col 64 = 1.0
            vp = [vpp.tile([128, TT, 2, 65], BF16, tag=f"v{p}", name=f"vp{p}")
                  for p in range(NPAIR)]
            for p in range(NPAIR):
                nc.vector.memset(vp[p][:, :, :, 64:65], 1.0)

            # ---------------- input DMA (consumption order) -------------------
            wk_sb = [wpool.tile([128, 256], BF16, tag=f"wk{c}", name=f"wk{c}")
                     for c in range(CT)]
            wq_sb = [wpool.tile([128, 256], BF16, tag=f"wq{c}", name=f"wq{c}")
                     for c in range(CT)]
            wv_sb = [wpool.tile([128, 256], BF16, tag=f"wv{c}", name=f"wv{c}")
                     for c in range(CT)]
            xT = [xtp.tile([128, T], BF16, tag=f"xT{c}", name=f"xT{c}")
                  for c in range(CT)]
            for c in range(CT):
                nc.sync.dma_start(out=wk_sb[c][:], in_=wk_s[c * 128:(c + 1) * 128, :])
                nc.sync.dma_start(out=wq_sb[c][:], in_=wq_s[c * 128:(c + 1) * 128, :])
                nc.sync.dma_start(out=xT[c][:, 0:1024],
                                  in_=xbT[c * 128:(c + 1) * 128, 0:1024])
            for c in range(CT):
                nc.sync.dma_start(out=xT[c][:, 1024:2048],
                                  in_=xbT[c * 128:(c + 1) * 128, 1024:2048])
            for c in range(CT):
                nc.sync.dma_start(out=wv_sb[c][:], in_=wv_s[c * 128:(c + 1) * 128, :])
            wo_sb = []
            for cp in range(NPAIR):
                wt = wpool.tile([128, D], BF16, tag=f"wo{cp}", name=f"wo{cp}")
                nc.gpsimd.dma_start(out=wt[:], in_=wo_s[cp * 128:(cp + 1) * 128, :])
                wo_sb.append(wt)
            ident = consts.tile([128, 128], BF16, tag="ident")
            nc.gpsimd.dma_start(out=ident[:], in_=ident_d)
            bd2 = consts.tile([128, 2], BF16, tag="bd2")
            nc.gpsimd.dma_start(out=bd2[:], in_=bd2_d)
            wqc = consts.tile([128, 1], F32, tag="wqc")
            nc.gpsimd.dma_start(out=wqc[:], in_=wqc_d)
            wkc = consts.tile([128, 1], F32, tag="wkc")
            nc.gpsimd.dma_start(out=wkc[:], in_=wkc_d)
            epsc = consts.tile([128, 1], F32, tag="epsc")
            nc.vector.memset(epsc[:], EPS)
            sel_sb = []
            for p in range(NPAIR):
                st = consts.tile([128, 128], BF16, tag=f"sel{p}", name=f"sel{p}")
                nc.gpsimd.dma_start(out=st[:], in_=sel_d[p])
                sel_sb.append(st)

            qt = {}      # (proj, pair) -> [128, T] bf16 staging
            ms = {}      # (proj, pair) -> [2, T] bf16 sumsq rows
            vt_sb = {}   # pair -> [128, T] bf16 V^T staging
            rstdD = {}   # (proj, pair) -> DRAM [2, T] bf16
            msD = {}
            for proj in ("k", "q"):
                for p in range(NPAIR):
                    qt[(proj, p)] = qtp.tile([128, T], BF16, tag=f"qt{proj}{p}",
                                             name=f"qt{proj}{p}")
                    ms[(proj, p)] = msp.tile([2, T], BF16, tag=f"ms{proj}{p}",
                                             name=f"ms{proj}{p}")
                    for qh in range(QH):
                        msD[(proj, p, qh)] = drp.tile(
                            [2, 1024], BF16, tag=f"msD{proj}{p}{qh}",
                            name=f"msD{proj}{p}{qh}")
                        rstdD[(proj, p, qh)] = drp.tile(
                            [2, 1024], BF16, tag=f"rsD{proj}{p}{qh}",
                            name=f"rsD{proj}{p}{qh}")
            for p in range(NPAIR):
                vt_sb[p] = qtp.tile([128, T], BF16, tag=f"vt{p}", name=f"vt{p}")

            ohp = [ohpp.tile([128, T], BF16, tag=f"ohp{p}", name=f"ohp{p}")
                   for p in range(NPAIR)]
            ohr = [ohpp.tile([128, T], BF16, tag=f"ohr{p}", name=f"ohr{p}")
                   for p in range(NPAIR)]


            W_SB = {"k": wk_sb, "q": wq_sb, "v": wv_sb}

            # ---------------- building blocks ---------------------------------
            def proj_chunk(proj, pair, qh, qq):
                dest = vt_sb[pair] if proj == "v" else qt[(proj, pair)]
                pj = psX.tile([128, 512], F32, tag="x")
                off = qh * 1024 + qq * 512
                for ct in range(CT):
                    nc.tensor.matmul(
                        pj[:], W_SB[proj][ct][:, pair * 128:(pair + 1) * 128],
                        xT[ct][:, off:off + 512],
                        start=(ct == 0), stop=(ct == CT - 1))
                with nc.allow_low_precision(reason="bf16 staging"):
                    nc.vector.tensor_copy(out=dest[:, off:off + 512], in_=pj[:])

            def sumsq_chunk(proj, pair, qh):
                q2 = q2p.tile([128, 1024], BF16, tag="q2")
                sl = slice(qh * 1024, (qh + 1) * 1024)
                with nc.allow_low_precision(reason="bf16 sumsq"):
                    nc.vector.tensor_tensor(out=q2[:], in0=qt[(proj, pair)][:, sl],
                                            in1=qt[(proj, pair)][:, sl], op=mult)
                for qq in range(2):
                    ss = psX.tile([128, 512], F32, tag="x")
                    nc.tensor.matmul(ss[0:2, :], bd2[:], q2[:, qq * 512:(qq + 1) * 512],
                                     start=True, stop=True)
                    # ACT is idle pre-attention; Copy lives in every act table
                    with nc.allow_low_precision(reason="bf16 ms"):
                        nc.scalar.copy(
                            ms[(proj, pair)][:, qh * 1024 + qq * 512:
                                             qh * 1024 + (qq + 1) * 512],
                            ss[0:2, :])

            def rstd_unit(proj, pair, qh):
                """rstdD = (ms/64+eps)^-1/2 for one query half, DMA-packed
                into [64,32] so the DVE chain is cheap"""
                sl = slice(qh * 1024, (qh + 1) * 1024)
                nm = f"{proj}{pair}{qh}"
                nc.sync.dma_start(out=msD[(proj, pair, qh)][:],
                                  in_=ms[(proj, pair)][:, sl])
                mp = packp.tile([64, 32], BF16, tag="pk16", name=f"mp{nm}")
                nc.sync.dma_start(out=mp[:], in_=dram_view(msD[(proj, pair, qh)], [64, 32]))
                m = packp.tile([64, 32], F32, tag="pk32a", name=f"m{nm}")
                mh = packp.tile([64, 32], F32, tag="pk32b", name=f"mh{nm}")
                y = packp.tile([64, 32], F32, tag="pk32c", name=f"y{nm}")
                t1 = packp.tile([64, 32], F32, tag="pk32d", name=f"t1{nm}")
                t2 = packp.tile([64, 32], F32, tag="pk32e", name=f"t2{nm}")
                rs = packp.tile([64, 32], BF16, tag="pk16b", name=f"rs{nm}")
                ts = nc.vector.tensor_scalar
                tt = nc.vector.tensor_tensor
                with nc.allow_low_precision(reason="rstd chain"):
                    ts(out=m[:], in0=mp[:], scalar1=1.0 / HD, scalar2=EPS,
                       op0=mult, op1=add)
                    ts(out=mh[:], in0=m[:], scalar1=0.5, scalar2=0.0,
                       op0=mult, op1=bypass)
                    # y0 = bitcast(((~bits) >> 1) - 0x20A8C620)
                    ts(out=y[:].bitcast(U32), in0=m[:].bitcast(U32),
                       scalar1=0xFFFFFFFF, scalar2=1, op0=bxor, op1=shr)
                    ts(out=y[:].bitcast(U32), in0=y[:].bitcast(U32),
                       scalar1=0x20A8C620, scalar2=0, op0=sub, op1=bypass)
                    for _ in range(2):  # two sign-cancelling Newton steps
                        tt(out=t1[:], in0=mh[:], in1=y[:], op=mult)
                        tt(out=t2[:], in0=t1[:], in1=y[:], op=mult)
                        ts(out=t2[:], in0=t2[:], scalar1=1.5, scalar2=0.0,
                           op0=sub, op1=bypass)
                        tt(out=y[:], in0=t2[:], in1=y[:], op=mult)
                    nc.vector.tensor_copy(out=rs[:], in_=y[:])
                nc.sync.dma_start(out=dram_view(rstdD[(proj, pair, qh)], [64, 32]),
                                  in_=rs[:])

            def rstd_unit_act(proj, pair, qh):
                """pair-0 warmup variant: rstd on the idle ACT engine via
                Ln/Exp in [2,1024] layout - skips the DMA pack round-trip.
                Must be emitted before the first attention exp (table order)."""
                sl = slice(qh * 1024, (qh + 1) * 1024)
                nm = f"{proj}{pair}{qh}"
                lnt = packp.tile([2, 1024], F32, tag="lnt", name=f"ln{nm}")
                rsa = packp.tile([2, 1024], BF16, tag="rsa", name=f"rsa{nm}")
                nc.scalar.activation(lnt[:], ms[(proj, pair)][:, sl], Ln,
                                     scale=1.0 / HD, bias=epsc[0:2, :])
                with nc.allow_low_precision(reason="bf16 rstd"):
                    nc.scalar.activation(rsa[:], lnt[:], Exp, scale=-0.5)
                nc.sync.dma_start(out=rstdD[(proj, pair, qh)][:], in_=rsa[:])

            def norm_chunk(proj, pair, qh):
                """qhat/khat[2p+i][64i:64i+64, :] = qt * wcol * rstd (bf16)"""
                dest = qhat if proj == "q" else khat
                wcol = wqc if proj == "q" else wkc
                rw = rwp.tile([128, 1024], BF16, tag="rwn")
                sl = slice(qh * 1024, (qh + 1) * 1024)
                for i in range(2):
                    nc.sync.dma_start(
                        out=rw[64 * i:64 * i + 64, :],
                        in_=bcast_row(rstdD[(proj, pair, qh)][i:i + 1, :]))
                with nc.allow_low_precision(reason="bf16 qkhat"):
                    for i in range(2):
                        rows = slice(64 * i, 64 * i + 64)
                        nc.vector.scalar_tensor_tensor(
                            out=dest[pair * 2 + i][rows, sl],
                            in0=qt[(proj, pair)][rows, sl],
                            scalar=wcol[rows, :],
                            in1=rw[rows, :],
                            op0=mult, op1=mult)

            def vtrans_chunk(pair, tq):
                """vt_sb[pair] kt 4tq..4tq+3 -> vp[pair] bf16 [V|1] weights"""
                vb = psX.tile([128, 512], F32, tag="x")
                vbb = vb[:].bitcast(BF16)  # [128, 1024] bf16 view
                for b_ in range(4):
                    kt = tq * 4 + b_
                    nc.tensor.transpose(vbb[:, b_ * 128:(b_ + 1) * 128],
                                        vt_sb[pair][:, kt * 128:(kt + 1) * 128],
                                        ident[:])
                for b_ in range(4):
                    kt = tq * 4 + b_
                    with nc.allow_low_precision(reason="bf16 v"):
                        nc.vector.tensor_copy(
                            out=vp[pair][:, kt, :, 0:64],
                            in_=vbb[:, b_ * 128:(b_ + 1) * 128].rearrange(
                                "p (h d) -> p h d", h=2))

            # ---------------- pre-attention: pair-0 K/Q pipeline --------------
            # per query-half so the first scores tile is gated by only half of
            # the projection work; V and its transposes hide under attention
            for qh in range(QH):
                for qq in range(2):
                    proj_chunk("k", 0, qh, qq)
                sumsq_chunk("k", 0, qh)
                rstd_unit_act("k", 0, qh)
                for qq in range(2):
                    proj_chunk("q", 0, qh, qq)
                sumsq_chunk("q", 0, qh)
                rstd_unit_act("q", 0, qh)
                norm_chunk("k", 0, qh)
                norm_chunk("q", 0, qh)
            for qh in range(QH):
                for qq in range(2):
                    proj_chunk("v", 0, qh, qq)

            # pair-1 work, finely chunked, interleaved under heads 0-1 of the
            # first query half; fully drained before head 2 is emitted
            bg = []
            for proj in ("k", "q"):
                for qh in range(QH):
                    for qq in range(2):
                        bg.append(lambda proj=proj, qh=qh, qq=qq:
                                  proj_chunk(proj, 1, qh, qq))
                    bg.append(lambda proj=proj, qh=qh: sumsq_chunk(proj, 1, qh))
                    bg.append(lambda proj=proj, qh=qh: rstd_unit(proj, 1, qh))
            for proj in ("k", "q"):
                for qh in range(QH):
                    bg.append(lambda proj=proj, qh=qh: norm_chunk(proj, 1, qh))
            for qh in range(QH):
                for qq in range(2):
                    bg.append(lambda qh=qh, qq=qq: proj_chunk("v", 1, qh, qq))
            for tq in range(4):
                bg.append(lambda tq=tq: vtrans_chunk(1, tq))

            def pop_bg(n=1):
                for _ in range(n):
                    if bg:
                        bg.pop(0)()

            # V-of-pair-0 transposes hide under head 0's first score tiles;
            # its O matmuls lag 4 tiles behind the exps
            bg0 = []
            for tq in range(4):
                bg0.append(lambda tq=tq: vtrans_chunk(0, tq))

            # ---------------- attention + output projection -------------------
            # pair-outer head order: pair-1's projection chain hides under all
            # four pair-0 (head, half) tiles instead of just two; the qh0
            # finalization (recip + output projection) pops inside the last
            # two heads' loops so it never stalls the PE queue
            dns = {qh: packp.tile([HPC, 1024], BF16, tag="dns", bufs=2,
                                  name=f"dns{qh}") for qh in range(QH)}

            def recip_ohr(qh):
                dnf = packp.tile([HPC, 1024], F32, tag="dnf", name=f"dnf{qh}")
                rcf = packp.tile([HPC, 1024], F32, tag="rcf", name=f"rcf{qh}")
                rc16 = packp.tile([128, 1024], BF16, tag="rc16", name=f"rc16{qh}")
                nc.gpsimd.memset(rc16[:], 0.0)
                with nc.allow_low_precision(reason="recip"):
                    nc.vector.tensor_copy(out=dnf[:], in_=dns[qh][:])
                    nc.vector.reciprocal_approx_fast(out=rcf[:], in_=dnf[:])
                    nc.vector.tensor_copy(out=rc16[0:HPC, :], in_=rcf[:])
                sl = slice(qh * 1024, (qh + 1) * 1024)
                for pair in range(NPAIR):
                    for qq in range(2):
                        rb = psX.tile([128, 512], F32, tag="x")
                        nc.tensor.matmul(rb[:], sel_sb[pair][:],
                                         rc16[:, qq * 512:(qq + 1) * 512],
                                         start=True, stop=True)
                        csl = slice(qh * 1024 + qq * 512, qh * 1024 + (qq + 1) * 512)
                        with nc.allow_low_precision(reason="bf16 ohat"):
                            nc.vector.tensor_tensor(out=ohr[pair][:, csl],
                                                    in0=ohp[pair][:, csl],
                                                    in1=rb[:], op=mult)

            def outproj_psx(qh, et):
                for qq in range(2):
                    ops = psX.tile([128, 512], F32, tag="x")
                    for cp in range(NPAIR):
                        nc.tensor.matmul(
                            ops[:], wo_sb[cp][:, et * 128:(et + 1) * 128],
                            ohr[cp][:, qh * 1024 + qq * 512:
                                    qh * 1024 + (qq + 1) * 512],
                            start=(cp == 0), stop=(cp == NPAIR - 1))
                    osb = osbp.tile([128, 512], BF16, tag="osb")
                    with nc.allow_low_precision(reason="bf16 out"):
                        if qq == 0:
                            nc.vector.tensor_copy(out=osb[:], in_=ops[:])
                        else:
                            nc.scalar.copy(osb[:], ops[:])
                    eng = (nc.gpsimd, nc.sync)[(et + qq) % 2]
                    eng.dma_start(
                        out=outT[et * 128:(et + 1) * 128,
                                 qh * 1024 + qq * 512:qh * 1024 + (qq + 1) * 512],
                        in_=osb[:])

            bg2 = []
            seq = [(0, 0), (1, 0), (0, 1), (1, 1), (2, 0), (3, 0), (2, 1), (3, 1)]
            for idx, (h, qh) in enumerate(seq):
                pair, i = h // 2, h % 2
                lag = 4 if idx == 0 else 1
                o_ps = psO.tile([65, 1024], F32, tag="o")
                pbs = {}

                def emit_o(kt, o_ps=o_ps, pbs=pbs, pair=pair, i=i):
                    for qq in range(2):
                        nc.tensor.matmul(
                            o_ps[:, qq * 512:(qq + 1) * 512],
                            vp[pair][:, kt, i, :],
                            pbs[kt][:, qq * 512:(qq + 1) * 512],
                            start=(kt == 0), stop=(kt == TT - 1))

                for kt in range(TT):
                    s_ps = psS.tile([128, 1024], F32, tag="s")
                    for qq in range(2):
                        nc.tensor.matmul(
                            s_ps[:, qq * 512:(qq + 1) * 512],
                            khat[h][:, kt * 128:(kt + 1) * 128],
                            qhat[h][:, qh * 1024 + qq * 512:
                                    qh * 1024 + (qq + 1) * 512],
                            start=True, stop=True)
                    pb = pbp.tile([128, 1024], BF16, tag="pb")
                    pbs[kt] = pb
                    nc.scalar.activation(pb[:], s_ps[:], Exp, scale=0.125)
                    if idx == 0 and bg0:
                        bg0.pop(0)()
                    if kt >= lag:
                        emit_o(kt - lag)
                    if pair == 0 and not bg0 and kt % 2 == 0:
                        pop_bg(1)
                    if idx >= 6 and bg2:
                        bg2.pop(0)()
                for kt in range(TT - lag, TT):
                    emit_o(kt)
                # evict unnormalized O + denominator row
                sl = slice(qh * 1024, (qh + 1) * 1024)
                dnt = dntp.tile([65, 1024], BF16, tag="dnt")
                with nc.allow_low_precision(reason="bf16 o"):
                    nc.vector.tensor_copy(out=dnt[:], in_=o_ps[:])
                nc.gpsimd.dma_start(out=dns[qh][h:h + 1, :], in_=dnt[64:65, :])
                nc.gpsimd.dma_start(out=ohp[pair][64 * i:64 * i + 64, sl],
                                    in_=dnt[0:64, :])
                if idx == 3:
                    pop_bg(len(bg))  # pair-1 data must exist before head 2
                if idx == 5:
                    bg2.append(lambda: recip_ohr(0))
                    for et in range(D // 128):
                        bg2.append(lambda et=et: outproj_psx(0, et))
            while bg2:
                bg2.pop(0)()
            # qh1 finalize at the tail, through the now-idle psS pool
            recip_ohr(1)
            for et in range(D // 128):
                ops = psS.tile([128, 1024], F32, tag="s")
                for qq in range(2):
                    for cp in range(NPAIR):
                        nc.tensor.matmul(
                            ops[:, qq * 512:(qq + 1) * 512],
                            wo_sb[cp][:, et * 128:(et + 1) * 128],
                            ohr[cp][:, 1024 + qq * 512:1024 + (qq + 1) * 512],
                            start=(cp == 0), stop=(cp == NPAIR - 1))
                for qq in range(2):
                    osb = osbp.tile([128, 512], BF16, tag="osb")
                    with nc.allow_low_precision(reason="bf16 out"):
                        if qq == 0:
                            nc.vector.tensor_copy(out=osb[:], in_=ops[:, 0:512])
                        else:
                            nc.scalar.copy(osb[:], ops[:, 512:1024])
                    eng = (nc.gpsimd, nc.sync, nc.scalar)[(2 * et + qq) % 3]
                    eng.dma_start(
                        out=outT[et * 128:(et + 1) * 128,
                                 1024 + qq * 512:1024 + (qq + 1) * 512],
                        in_=osb[:])

    nc.compile()
    return nc


def _get_compiled():
    global _COMPILED
    if _COMPILED is None:
        _COMPILED = _build()
    return _COMPILED


def kernel(x, wq, wk, wv, wo, q_norm_w, k_norm_w):
    import ml_dtypes
    from concourse.bass_utils import run_bass_kernel_spmd

    global LAST_EXEC_NS
    if os.environ.get("BASS_TRACE"):
        _install_ntff_shim()

    x = np.asarray(x, dtype=np.float32)
    wq = np.asarray(wq, dtype=np.float32)
    wk = np.asarray(wk, dtype=np.float32)
    wv = np.asarray(wv, dtype=np.float32)
    wo = np.asarray(wo, dtype=np.float32)
    q_norm_w = np.asarray(q_norm_w, dtype=np.float32)
    k_norm_w = np.asarray(k_norm_w, dtype=np.float32)

    nc = _get_compiled()
    bf = lambda a: a.astype(ml_dtypes.bfloat16)

    ident = np.eye(128, dtype=np.float32)
    bd2 = np.zeros((128, 2), np.float32)
    bd2[0:64, 0] = 1.0
    bd2[64:128, 1] = 1.0
    wqc = np.concatenate([q_norm_w, q_norm_w]).reshape(128, 1).astype(np.float32)
    wkc = np.concatenate([k_norm_w, k_norm_w]).reshape(128, 1).astype(np.float32)
    sels = []
    for p in range(NPAIR):
        sm = np.zeros((128, 128), np.float32)
        sm[2 * p, 0:64] = 1.0
        sm[2 * p + 1, 64:128] = 1.0
        sels.append(sm)

    in_maps = []
    for c in range(N_CORES):
        b = c // 4
        hs = HPC * (c % 4)
        # head split in reference is strided: head h uses columns d*H + h
        perm = ((hs + np.arange(HPC))[:, None] + H * np.arange(HD)[None, :]).reshape(-1)
        in_maps.append({
            "xbT": bf(np.ascontiguousarray(x[b].T)),
            "wq_s": bf(np.ascontiguousarray(wq[:, perm])),
            "wk_s": bf(np.ascontiguousarray(wk[:, perm])),
            "wv_s": bf(np.ascontiguousarray(wv[:, perm])),
            "wo_s": bf(np.ascontiguousarray(wo[hs * HD:(hs + HPC) * HD, :])),
            "ident": bf(ident), "bd2": bf(bd2),
            "wqc": wqc, "wkc": wkc,
            "sel0": bf(sels[0]), "sel1": bf(sels[1]),
        })

    res = run_bass_kernel_spmd(nc, in_maps, core_ids=list(range(N_CORES)),
                               trace=bool(os.environ.get("BASS_TRACE")),
                               tmpdir=os.environ.get("BASS_TRACE_DIR"))
    LAST_EXEC_NS = res.exec_time_ns

    out = np.empty((B, T, D), dtype=np.float32)
    for b in range(B):
        acc = res.results[4 * b]["outT"].astype(np.float32)
        for c in range(4 * b + 1, 4 * b + 4):
            acc = acc + res.results[c]["outT"].astype(np.float32)
        out[b] = acc.T
    return out
